# revision 21
# baseline (speedup 1.0000x reference)
"""Multi-head causal attention block (B=4, T=2048, C=1024, H=16) on 8 TRN2 cores.

Sharding: core c handles batch b = c // 2 and head-group hg = c % 2 (8 heads).
Each core computes q/k/v for its 8 heads from x[b], runs causal attention, and
produces a partial output-projection y_partial[b] = attnout @ out_w[rows_hg].
Host sums the two head-group partials per batch (fp32) and adds out_b.

All matmul operands are bf16 (fp32 matmul is 2-pass on the PE; bf16 is
single-pass => 2x tensor throughput), accumulation stays fp32 in PSUM.
Feature-major layout: x is fed as xT = x[b].T so the QKV projection needs no
on-device transposes; q/k come out d-major with head pairs stacked on
partitions 0-63 / 64-127 (row-packed K=64 score matmuls via tile_position).

V tiles carry 64 ones-columns per head ([128, 8*(64+64)], ones first): the
attn@V matmul then emits the softmax denominator pre-broadcast on output
partitions 0-63 and the raw output on 64-127, so normalization is a
reciprocal + multiply straight out of PSUM - no broadcast matmul, no den/raw
staging copies. The extra lhsT columns are free (matmul cost is
stream-length-driven), and the denominator lands base-partition-0, which the
custom-DVE reciprocal_approx_fast requires.

Schedule: attention starts as early as possible (first exp ~20us in). Every
projection unit (V token-blocks, Q/K chunks of all pairs, output-projection
halves) is a closure in a work dict; strip prerequisites are force-run
staggered (Q chunk at strip entry, the strip's K chunk right before its first
diagonal group, V block kb right before group kb, next strip's first unit at
strip end), and the rest drain as PE filler between attention groups under a
deficit regulator: each group accrues its measured exp-minus-PE time
(~470-680ns), saturating at ~2 groups (the score double-buffer depth), and a
unit is popped only when the local deficit covers its PE cost. This keeps the
ScalarE exp stream (~165us busy) overlapped with the PE stream (~196us of
matmul work) over nearly the whole kernel, keeps the PE dense enough that the
HAM clock gate stays at 2.4GHz, and leaves the leftover output-projection
units to drain PE-back-to-back at the tail.

DMAs are merged into few 3D transfers (each DMA_DIRECT2D costs ~0.65us of
Sync-engine issue time): per-pair Q/K weights are host-packed so one DMA
loads one pair's tensor, and y is written back one 128x1024 tile per
token-block.
"""

import os
import sys
from collections import deque
from contextlib import ExitStack

import numpy as np
import ml_dtypes

for _p in ("/opt/trn_rl_repo", "/root/.axon_site/_ro/trn_rl_repo"):
    if os.path.isdir(_p) and _p not in sys.path:
        sys.path.insert(0, _p)

import concourse.bass as bass
import concourse.bacc as bacc
import concourse.mybir as mybir
import concourse.tile as tile
from concourse.bass_utils import run_bass_kernel_spmd

B, T, C, H = 4, 2048, 1024, 16
D = C // H  # 64
N_CORES = 8
HG = 2  # head groups per batch (cores per batch)
HPG = H // HG  # 8 heads per core
PAIRS = HPG // 2  # 4 head pairs per core
TB = T // 128  # 16 token blocks
QT = T // 512  # 4 q tiles
CT = C // 128  # 8 contraction tiles
FP32 = mybir.dt.float32
BF16 = mybir.dt.bfloat16
BF16_NP = ml_dtypes.bfloat16
SCALE = 1.0 / np.sqrt(np.float32(D))

_program_cache = {}


def build_program():
    nc = bacc.Bacc("TRN2", target_bir_lowering=False, debug=False, num_devices=N_CORES)

    xT = nc.declare_dram_parameter("xT", [C, T], BF16, isOutput=False)
    # host-packed: col block pr*1024+ci*128+f holds wq[ci*128+p, pr*128+f]
    wq = nc.declare_dram_parameter("wq", [128, PAIRS * C], BF16, isOutput=False)
    wk = nc.declare_dram_parameter("wk", [128, PAIRS * C], BF16, isOutput=False)
    wv = nc.declare_dram_parameter("wv", [C, 512], BF16, isOutput=False)
    bqk = nc.declare_dram_parameter("bqk", [128, 2 * PAIRS], FP32, isOutput=False)
    bv = nc.declare_dram_parameter("bv", [1, 512], FP32, isOutput=False)
    wo = nc.declare_dram_parameter("wo", [512, C], BF16, isOutput=False)
    maskp = nc.declare_dram_parameter("mask", [128, 128], BF16, isOutput=False)
    y = nc.declare_dram_parameter("y", [T, C], BF16, isOutput=True)

    Exp = mybir.ActivationFunctionType.Exp

    with tile.TileContext(nc) as tc, ExitStack() as ctx:
        persist = ctx.enter_context(tc.tile_pool(name="persist", bufs=1))

        mask_sb = persist.tile([128, 128], BF16, name="mask_sb", tag="mask_sb")
        bqk_sb = persist.tile([128, 2 * PAIRS], FP32, name="bqk_sb", tag="bqk_sb")
        bq_sb = bqk_sb[:, 0:PAIRS]
        bk_sb = bqk_sb[:, PAIRS : 2 * PAIRS]

        # V with 64 ones-columns per head: group h holds V[:, h*64:(h+1)*64] | 1*64
        v_sb = [
            persist.tile([128, HPG * 128], BF16, name=f"v_sb{i}", tag=f"v_sb{i}")
            for i in range(TB)
        ]
        qst = [
            persist.tile([128, T], BF16, name=f"qst{p}", tag=f"qst{p}")
            for p in range(PAIRS)
        ]
        kst = [
            persist.tile([128, T], BF16, name=f"kst{p}", tag=f"kst{p}")
            for p in range(PAIRS)
        ]
        onorm = [
            persist.tile([128, T], BF16, name=f"onorm{p}", tag=f"on{p}")
            for p in range(PAIRS)
        ]
        wo_all = persist.tile([128, PAIRS, C], BF16, name="wo_all", tag="wo_all")
        # xT lives in 4 per-column-chunk tiles so consumers wait only on the
        # chunk they read
        xt_c = [
            persist.tile([128, CT, 512], BF16, name=f"xt_c{c}", tag=f"xt_c{c}")
            for c in range(T // 512)
        ]
        wv_all = persist.tile([128, CT, 512], BF16, name="wv_all", tag="wv_all")
        bv_bc = persist.tile([128, 512], FP32, name="bv_bc", tag="bv_bc")
        wq_sb = [
            persist.tile([128, CT, 128], BF16, name=f"wq_sb{p}", tag=f"wq_sb{p}")
            for p in range(PAIRS)
        ]
        wk_sb = [
            persist.tile([128, CT, 128], BF16, name=f"wk_sb{p}", tag=f"wk_sb{p}")
            for p in range(PAIRS)
        ]

        def dma_w(dst, src, pr):
            nc.sync.dma_start(
                dst[pr],
                src[:, pr * C : (pr + 1) * C].rearrange("p (a f) -> p a f", f=128),
            )

        def dma_xt(c):
            step = 4
            for a0 in range(0, CT, step):
                nc.sync.dma_start(
                    xt_c[c][:, a0 : a0 + step, :],
                    xT[
                        a0 * 128 : (a0 + step) * 128, c * 512 : (c + 1) * 512
                    ].rearrange("(a p) f -> p a f", p=128),
                )

        # DMA issue order = first-need order (transfers share the ~320GB/s
        # aggregate and complete roughly in issue order, so the bootstrap set
        # must go first and the bulk weights last).
        # chunk-0 pieces are split fine so the first matmuls can start on
        # partial data while the rest streams in
        nc.sync.dma_start(
            wk_sb[0][:, 0:4, :],
            wk[:, 0:512].rearrange("p (a f) -> p a f", f=128),
        )
        nc.sync.dma_start(
            xt_c[0][:, 0:2, :],
            xT[0:256, 0:512].rearrange("(a p) f -> p a f", p=128),
        )
        nc.sync.dma_start(
            xt_c[0][:, 2:4, :],
            xT[256:512, 0:512].rearrange("(a p) f -> p a f", p=128),
        )
        nc.sync.dma_start(
            wk_sb[0][:, 4:8, :],
            wk[:, 512:1024].rearrange("p (a f) -> p a f", f=128),
        )
        nc.sync.dma_start(
            xt_c[0][:, 4:6, :],
            xT[512:768, 0:512].rearrange("(a p) f -> p a f", p=128),
        )
        nc.sync.dma_start(
            xt_c[0][:, 6:8, :],
            xT[768:1024, 0:512].rearrange("(a p) f -> p a f", p=128),
        )
        dma_w(wq_sb, wq, 0)
        nc.sync.dma_start(bqk_sb, bqk[:, :])
        for g in (0, 1):
            nc.sync.dma_start(
                wv_all[:, g * 4 : (g + 1) * 4, :],
                wv[g * 512 : (g + 1) * 512, :].rearrange("(a p) f -> p a f", p=128),
            )
        nc.sync.dma_start(bv_bc, bv[:, :].to_broadcast([128, 512]))
        nc.sync.dma_start(mask_sb, maskp[:, :])
        for c in range(1, T // 512):
            dma_xt(c)
            dma_w(wq_sb, wq, c)
            dma_w(wk_sb, wk, c)
        for g in (0, 1):
            nc.sync.dma_start(
                wo_all[:, g * 2 : (g + 1) * 2, :],
                wo[g * 256 : (g + 1) * 256, :].rearrange("(a p) f -> p a f", p=128),
            )

        # one-time ones-columns for the V tiles (DVE, off critical path)
        for tb in range(TB):
            vt = v_sb[tb].rearrange("p (h e) -> p h e", e=128)
            nc.vector.memset(vt[:, :, 0:64], 1.0)

        p_pool = ctx.enter_context(tc.tile_pool(name="pexp", bufs=10))
        small_pool = ctx.enter_context(tc.tile_pool(name="small", bufs=6))

        # PSUM budget (8 banks): sps 2x[128,1024]f32 = 4, outps 2x[128,512] = 2,
        # pqp (shared by V / QK-proj / out-proj units) 2x[128,512] = 2.
        spsum = ctx.enter_context(tc.tile_pool(name="spsum", bufs=2, space="PSUM"))
        apsum = ctx.enter_context(tc.tile_pool(name="apsum", bufs=1, space="PSUM"))
        pqp = ctx.enter_context(tc.tile_pool(name="pqp", bufs=2, space="PSUM"))

        # ---------------- work units (filler stream) ----------------
        units = {}

        def u_v(tb):
            def go():
                pv = pqp.tile([128, 512], FP32, name="pv", tag="pq")
                for ci in range(CT):
                    nc.tensor.matmul(
                        pv,
                        xt_c[tb // 4][:, ci, (tb % 4) * 128 : (tb % 4) * 128 + 128],
                        wv_all[:, ci, :],
                        start=(ci == 0),
                        stop=(ci == CT - 1),
                    )
                vt = v_sb[tb].rearrange("p (h e) -> p h e", e=128)
                nc.vector.tensor_add(
                    vt[:, :, 64:128],
                    pv.rearrange("p (h e) -> p h e", e=64),
                    bv_bc.rearrange("p (h e) -> p h e", e=64),
                )

            return go

        def u_qk(pr, which, qt):
            w_all = (wq_sb if which == "q" else wk_sb)[pr]
            bias_sb = bq_sb if which == "q" else bk_sb
            dst = (qst if which == "q" else kst)[pr]

            def go():
                pq = pqp.tile([128, 512], FP32, name="pq", tag="pq")
                for ci in range(CT):
                    nc.tensor.matmul(
                        pq,
                        w_all[:, ci, :],
                        xt_c[qt][:, ci, :],
                        start=(ci == 0),
                        stop=(ci == CT - 1),
                    )
                nc.vector.tensor_scalar_add(
                    dst[:, qt * 512 : (qt + 1) * 512],
                    pq,
                    bias_sb[:, pr : pr + 1],
                )

            return go

        ys_live = {}

        def u_outproj(tb, nh):
            def go():
                if tb in ys_live:
                    ys = ys_live.pop(tb)
                else:
                    ys = small_pool.tile([128, 1024], BF16, name="ys", tag="ys")
                    ys_live[tb] = ys
                yp = pqp.tile([128, 512], FP32, name="yp", tag="pq")
                for p2 in range(PAIRS):
                    nc.tensor.matmul(
                        yp,
                        onorm[p2][:, tb * 128 : (tb + 1) * 128],
                        wo_all[:, p2, nh * 512 : (nh + 1) * 512],
                        start=(p2 == 0),
                        stop=(p2 == PAIRS - 1),
                    )
                nc.vector.tensor_copy(ys[:, nh * 512 : (nh + 1) * 512], yp)
                if nh == 1:
                    nc.sync.dma_start(y[tb * 128 : (tb + 1) * 128, :], ys)

            return go

        for tb in range(TB):
            units[("v", tb)] = u_v(tb)
        for pr in range(PAIRS):
            for qt in range(QT):
                units[("k", pr, qt)] = u_qk(pr, "k", qt)
                units[("q", pr, qt)] = u_qk(pr, "q", qt)
        for tb in range(TB):
            for nh in (0, 1):
                units[("o", tb, nh)] = u_outproj(tb, nh)

        misc = deque()
        # filler pacing: per attention group the ScalarE exp runs ~0.4-0.5us
        # longer than the group's PE work; accumulate that deficit and emit
        # one ~1.7us filler unit per ~4 groups so the PE neither drowns in
        # projection work early (starving the exp stream) nor runs dry late.
        UNIT_NS = 1700
        # measured exp-minus-PE time per attention group, keyed by ncols
        GROUP_DEFICIT = {512: 472, 384: 676, 256: 620, 128: 574}
        state = {"deficit": 0.0}

        def unit_cost(key):
            return 854.0 if key[0] == "o" else float(UNIT_NS)

        def run_unit(key):
            go = units.pop(key, None)
            if go is not None:
                go()
                state["deficit"] = max(state["deficit"] - unit_cost(key), -1.0 * UNIT_NS)

        def accrue(ns):
            # local regulator, not a global ledger: ACT can only run ~2 groups
            # ahead of the PE (sps double-buffer), so surplus saturates; idle
            # ACT time must not bank unlimited credit either way.
            state["deficit"] = min(state["deficit"] + ns, 2.2 * UNIT_NS)

        def pop_misc():
            while misc:
                if misc[0] not in units:
                    misc.popleft()
                    continue
                if state["deficit"] < unit_cost(misc[0]):
                    return
                run_unit(misc.popleft())

        # ---------------- attention stream ----------------
        # prerequisites are force-run staggered: Q chunk at strip entry, the
        # strip's K chunk right before the first diagonal group, V block kb
        # right before group kb - so forced projection work never bunches up
        # with the exp stream idle.
        pending = None
        for pr in range(PAIRS):
            if pr + 1 < PAIRS:
                for qt in range(QT):
                    misc.append(("k", pr + 1, qt))
                    misc.append(("q", pr + 1, qt))
            for qt in range(QT):
                if qt == 0:
                    run_unit(("k", pr, 0))
                run_unit(("q", pr, qt))
                for c in range(qt):
                    run_unit(("k", pr, c))

                nkb = 4 * qt + 4
                outps = [
                    apsum.tile([128, 512], FP32, name=f"outp{hh}", tag=f"av{hh}")
                    for hh in (0, 1)
                ]
                # 1-k-block groups: off-diagonal (full 512 q cols), then the 4
                # diagonal sub-blocks (column-trimmed). col1 = hh1's column
                # offset in the score/pexp tile - always 512 (bank 1): the two
                # head-halves run CONCURRENTLY on different PE row groups, so
                # they must drain into different PSUM banks.
                subs = [(kb, 512, 0, 512, False) for kb in range(4 * qt)]
                subs += [
                    (4 * qt + j, 512 - 128 * j, 128 * j, 512, True) for j in range(4)
                ]

                for si, (kb, ncols, qcol0, col1, diag) in enumerate(subs):
                    if kb == 4 * qt and qt > 0:
                        run_unit(("k", pr, qt))
                    sps = spsum.tile([128, 1024], FP32, name="sps", tag="sc")
                    for hh in (0, 1):
                        c0 = hh * col1
                        nc.tensor.matmul(
                            sps[:, c0 : c0 + ncols],
                            kst[pr][hh * 64 : hh * 64 + 64, kb * 128 : (kb + 1) * 128],
                            qst[pr][
                                hh * 64 : hh * 64 + 64,
                                qt * 512 + qcol0 : qt * 512 + qcol0 + ncols,
                            ],
                            start=True,
                            stop=True,
                            tile_position=(hh * 64, 0),
                        )
                    pexp = p_pool.tile([128, 1024], BF16, name="pexp", tag="p")
                    if col1 == ncols or ncols == 512:
                        nc.scalar.activation(
                            pexp[:, 0 : col1 + ncols],
                            sps[:, 0 : col1 + ncols],
                            Exp,
                            scale=float(SCALE),
                        )
                    else:
                        for hh in (0, 1):
                            c0 = hh * col1
                            nc.scalar.activation(
                                pexp[:, c0 : c0 + ncols],
                                sps[:, c0 : c0 + ncols],
                                Exp,
                                scale=float(SCALE),
                            )
                    if diag:
                        # zero the strictly-upper triangle of the 128-wide
                        # diagonal window (post-exp 0/1 mask)
                        for hh in (0, 1):
                            c0 = hh * col1
                            nc.vector.tensor_mul(
                                pexp[:, c0 : c0 + 128],
                                pexp[:, c0 : c0 + 128],
                                mask_sb,
                            )
                    if pending is not None:
                        pending()

                    def attnv(
                        pexp=pexp,
                        kb=kb,
                        ncols=ncols,
                        qcol0=qcol0,
                        col1=col1,
                        outps=outps,
                        pr=pr,
                        first=(kb == 0),
                        last=(kb == nkb - 1),
                    ):
                        vs = v_sb[kb].rearrange("p (h e) -> p h e", e=128)
                        for hh in (0, 1):
                            c0 = hh * col1
                            nc.tensor.matmul(
                                outps[hh][:, qcol0 : qcol0 + ncols],
                                vs[:, 2 * pr + hh, :],
                                pexp[:, c0 : c0 + ncols],
                                start=first,
                                stop=last,
                            )

                    pending = attnv
                    if pr == 0:
                        run_unit(("v", kb))
                    accrue(GROUP_DEFICIT[ncols])
                    pop_misc()

                # prefetch the next strip's first-needed projection unit while
                # the last group's exp completes, then flush the strip's last
                # attn@V and normalize from PSUM: output partitions 0-63 hold
                # the denominator pre-broadcast.
                nxt_pr, nxt_qt = (pr, qt + 1) if qt + 1 < QT else (pr + 1, 0)
                if nxt_pr < PAIRS:
                    if nxt_qt == 0:
                        run_unit(("k", nxt_pr, 0))
                    else:
                        run_unit(("q", nxt_pr, nxt_qt))
                pop_misc()
                pending()
                pending = None
                for hh in (0, 1):
                    # den sits on partitions 0-63 (ones-first V layout): the
                    # custom-DVE reciprocal needs a partition-0-based input
                    rbc = small_pool.tile([64, 512], FP32, name="rbc", tag="rbc")
                    nc.vector.reciprocal_approx_fast(rbc, outps[hh][0:64, :])
                    nc.vector.tensor_mul(
                        onorm[pr][hh * 64 : hh * 64 + 64, qt * 512 : (qt + 1) * 512],
                        outps[hh][64:128, :],
                        rbc,
                    )
                pop_misc()
                if pr == PAIRS - 1:
                    for tb in range(qt * 4, qt * 4 + 4):
                        for nh in (0, 1):
                            misc.append(("o", tb, nh))

        while misc:
            run_unit(misc.popleft())
        # safety net: run anything never reached
        for key in list(units):
            run_unit(key)

    if not nc.is_finalized():
        nc.finalize()
    return nc


def shard_inputs(x, qkv_w, qkv_b, out_w):
    """Build the 8 per-core input maps (host-side bf16 casts + packing)."""
    x = np.asarray(x, dtype=np.float32)
    qkv_w = np.asarray(qkv_w, dtype=np.float32)
    qkv_b = np.asarray(qkv_b, dtype=np.float32)
    out_w = np.asarray(out_w, dtype=np.float32)

    # 0/1 lower-triangular keep-mask for the post-exp diagonal-window zeroing
    mask = (np.arange(128)[:, None] <= np.arange(128)[None, :]).astype(BF16_NP)

    def pack_w(w512):
        # [C, 512] -> [128, PAIRS*C]: col pr*C + ci*128 + f  <-  w512[ci*128+p, pr*128+f]
        w = w512.reshape(CT, 128, PAIRS, 128)  # (ci, p, pr, f)
        w = w.transpose(1, 2, 0, 3).reshape(128, PAIRS * C)
        return np.ascontiguousarray(w).astype(BF16_NP)

    in_maps = []
    for core in range(N_CORES):
        b, hg = core // HG, core % HG
        col0 = hg * 512
        wq_np = pack_w(qkv_w[:, col0 : col0 + 512])
        wk_np = pack_w(qkv_w[:, C + col0 : C + col0 + 512])
        wv_np = np.ascontiguousarray(
            qkv_w[:, 2 * C + col0 : 2 * C + col0 + 512]
        ).astype(BF16_NP)
        bq_np = qkv_b[col0 : col0 + 512].reshape(PAIRS, 128).T
        bk_np = qkv_b[C + col0 : C + col0 + 512].reshape(PAIRS, 128).T
        bqk_np = np.ascontiguousarray(np.concatenate([bq_np, bk_np], axis=1))
        bv_np = np.ascontiguousarray(
            qkv_b[2 * C + col0 : 2 * C + col0 + 512].reshape(1, 512)
        )
        wo_np = np.ascontiguousarray(out_w[col0 : col0 + 512, :]).astype(BF16_NP)
        xT_np = np.ascontiguousarray(x[b].T).astype(BF16_NP)
        in_maps.append(
            {
                "xT": xT_np,
                "wq": wq_np,
                "wk": wk_np,
                "wv": wv_np,
                "bqk": bqk_np,
                "bv": bv_np,
                "wo": wo_np,
                "mask": mask,
            }
        )
    return in_maps


def kernel(x, qkv_w, qkv_b, out_w, out_b, _trace=False, _tmpdir=None):
    if "nc" not in _program_cache:
        _program_cache["nc"] = build_program()
    nc = _program_cache["nc"]

    in_maps = shard_inputs(x, qkv_w, qkv_b, out_w)
    res = run_bass_kernel_spmd(
        nc,
        in_maps,
        core_ids=list(range(N_CORES)),
        trace=_trace,
        tmpdir=_tmpdir,
    )
    _program_cache["last_results"] = res

    out_b = np.asarray(out_b, dtype=np.float32)
    y = np.empty((B, T, C), dtype=np.float32)
    for b in range(B):
        y[b] = (
            res.results[2 * b]["y"].astype(np.float32)
            + res.results[2 * b + 1]["y"].astype(np.float32)
            + out_b
        )
    return y
